# revision 6
# baseline (speedup 1.0000x reference)
"""Axial attention TRN2 kernel v2: 8-core SPMD, group-phased pipeline.

Same sharding as v1 (rows i-sharded, cols j-sharded, host gathers), but the
per-core schedule is restructured around groups of 4 sequences with a
3-stage software pipeline (stats(g+1) | body(g) | tail(g-1)):
 - LN sqrt batched once per group (2 act-table loads per group, not per seq)
 - bias matmuls elided when the folded biases are all zero
 - RoPE rotation via DVE stream_shuffle (adjacent-partition swap; the sign
   lives in the sin constant) instead of a PE permutation matmul
 - reciprocal broadcast round-trip batched to 2 SWDGE descriptors per group
 - elementwise work spread across DVE / Act / Pool to keep all engines near
   the PE's ~4.5us/seq; PSUM = 2x1-bank rotating + 2x2-bank scores +
   2-bank sums = 8 banks
"""
import sys
import numpy as np

sys.path.insert(0, "/opt/trn_rl_repo")

import ml_dtypes  # noqa: E402

import concourse.bass as bass  # noqa: E402
import concourse.bacc as bacc  # noqa: E402
import concourse.mybir as mybir  # noqa: E402
import concourse.tile as tile  # noqa: E402
from concourse.bass_utils import run_bass_kernel_spmd  # noqa: E402

F32 = mybir.dt.float32
BF16 = mybir.dt.bfloat16
BF = ml_dtypes.bfloat16

B, I, J, DIM, IDIM, HEADS = 1, 256, 256, 256, 64, 4
NCORES = 8
NROW = I // NCORES
NCOL = J // NCORES
EPS = 1e-5

PCOL = (0, 2, 1, 3)
Act = mybir.ActivationFunctionType
Alu = mybir.AluOpType
SWAP_MASK = [i ^ 1 for i in range(32)]


def _build_nc(n_row, n_col, with_bias):
    nc = bacc.Bacc("TRN2", target_bir_lowering=False, debug=True)

    xr_in = nc.declare_dram_parameter("xr", [n_row, 256, 256], F32, isOutput=False)
    xc_in = nc.declare_dram_parameter("xc", [n_col, 256, 256], F32, isOutput=False)
    yr_out = nc.declare_dram_parameter("yr", [n_row, 256, 256], F32, isOutput=True)
    yc_out = nc.declare_dram_parameter("yc", [n_col, 256, 256], F32, isOutput=True)

    wp = {}
    for w in ("a", "b"):
        for nm in ("wq", "wk", "wv", "wo"):
            wp[f"{nm}_{w}"] = nc.declare_dram_parameter(
                f"{nm}_{w}", [2, 128, 256], BF16, isOutput=False)
        if with_bias:
            for nm in ("bq", "bk", "bv", "bo"):
                wp[f"{nm}_{w}"] = nc.declare_dram_parameter(
                    f"{nm}_{w}", [1, 256], BF16, isOutput=False)
        for nm in ("cos", "sin"):
            wp[f"{nm}_{w}"] = nc.declare_dram_parameter(
                f"{nm}_{w}", [128, 512], BF16, isOutput=False)
    r2_in = nc.declare_dram_parameter("r2", [128, 128], BF16, isOutput=False)
    idt_in = nc.declare_dram_parameter("idt", [128, 128], BF16, isOutput=False)

    n_seq = n_row + n_col
    assert n_seq % 4 == 0 and n_row % 4 == 0
    n_grp = n_seq // 4
    rec_dram = nc.dram_tensor("rec_scratch", [n_grp, 4, 1024], BF16)

    with tile.TileContext(nc) as tc:
        with tc.tile_pool(name="const", bufs=1) as cp, \
             tc.tile_pool(name="work", bufs=4) as wk, \
             tc.tile_pool(name="hold", bufs=4) as hp, \
             tc.tile_pool(name="psA", bufs=3, space="PSUM") as ps_a, \
             tc.tile_pool(name="psS", bufs=3, space="PSUM") as ps_s, \
             tc.tile_pool(name="psU", bufs=1, space="PSUM") as ps_u:

            const = {}
            for w in ("a", "b"):
                for nm in ("wq", "wk", "wv", "wo"):
                    t = cp.tile([128, 2, 256], BF16, tag=f"{nm}_{w}",
                                name=f"{nm}_{w}_t")
                    nc.sync.dma_start(
                        out=t, in_=wp[f"{nm}_{w}"][:].rearrange("a p d -> p a d"))
                    const[f"{nm}_{w}"] = t
                if with_bias:
                    for nm in ("bq", "bk", "bv", "bo"):
                        t = cp.tile([1, 256], BF16, tag=f"{nm}_{w}",
                                    name=f"{nm}_{w}_t")
                        nc.sync.dma_start(out=t, in_=wp[f"{nm}_{w}"][:])
                        const[f"{nm}_{w}"] = t
                for nm in ("cos", "sin"):
                    t = cp.tile([128, 512], BF16, tag=f"{nm}_{w}",
                                name=f"{nm}_{w}_t")
                    nc.sync.dma_start(out=t, in_=wp[f"{nm}_{w}"][:])
                    const[f"{nm}_{w}"] = t
            r2 = cp.tile([128, 128], BF16, tag="r2")
            nc.sync.dma_start(out=r2, in_=r2_in[:])
            idt = cp.tile([128, 128], BF16, tag="idt")
            nc.sync.dma_start(out=idt, in_=idt_in[:])
            ones_col = cp.tile([128, 1], BF16, tag="ones_col")
            nc.vector.memset(ones_col, 1.0)
            ones_row = cp.tile([1, 256], BF16, tag="ones_row")
            nc.vector.memset(ones_row, 1.0)
            eps_t = cp.tile([128, 1], F32, tag="eps")
            nc.vector.memset(eps_t, EPS)

            xt_s = {}
            osb_s = {}
            grp_stats = {}
            grp_rec = {}
            grp_sums = {}

            def seq_info(s):
                is_row = s < n_row
                return (is_row, s if is_row else s - n_row,
                        "a" if is_row else "b",
                        xr_in if is_row else xc_in,
                        yr_out if is_row else yc_out)

            def load_grp(g):
                """Issue the 4 input DMAs for group g."""
                for lane in range(4):
                    s = 4 * g + lane
                    _, si, _, xin, _ = seq_info(s)
                    xt = hp.tile([128, 512], F32, tag="xt", bufs=20,
                                 name=f"xt{s}")
                    nc.sync.dma_start(
                        out=xt.rearrange("p (a d) -> p a d", a=2),
                        in_=xin[si].rearrange("(a p) d -> p a d", p=128))
                    xt_s[s] = xt

            def stats_grp(g):
                """LN stats for 4 seqs; one batched sqrt+recip."""
                mvg = wk.tile([128, 16], F32, tag="mvg", bufs=3, name=f"mvg{g}")
                for lane in range(4):
                    s = 4 * g + lane
                    xt = xt_s[s]
                    for tb in range(2):
                        st = wk.tile([128, 6], F32, tag="st", name=f"st{s}_{tb}")
                        nc.vector.bn_stats(st, xt[:, tb * 256:(tb + 1) * 256])
                        c = lane * 4 + tb * 2
                        nc.vector.bn_aggr(mvg[:, c:c + 2], st)
                # vars live at odd columns of mvg; sqrt(var+eps) batched
                var_ap = bass.AP(tensor=mvg.tensor, offset=mvg.offset + 1,
                                 ap=[[mvg.ap[0][0], 128], [2, 8]])
                sdg = wk.tile([128, 8], F32, tag="sdg", bufs=3, name=f"sdg{g}")
                nc.scalar.activation(sdg, var_ap, Act.Sqrt, bias=eps_t)
                inv = wk.tile([128, 8], F32, tag="inv", bufs=3, name=f"inv{g}")
                nc.vector.reciprocal(inv, sdg)
                grp_stats[g] = (mvg, inv)

            grp_front = {}

            def front(g):
                """LN apply -> transpose -> q/k proj -> rope for 4 seqs."""
                mvg, inv = grp_stats.pop(g)

                # wave 1: LN apply for all lanes
                xn_l = []
                for lane in range(4):
                    s = 4 * g + lane
                    xt = xt_s[s]
                    xn = wk.tile([128, 512], BF16, tag="xn", bufs=5,
                                 name=f"xn{s}")
                    for tb in range(2):
                        sl = slice(tb * 256, (tb + 1) * 256)
                        c = lane * 4 + tb * 2
                        nc.vector.tensor_scalar(
                            out=xn[:, sl], in0=xt[:, sl],
                            scalar1=mvg[:, c:c + 1],
                            scalar2=inv[:, lane * 2 + tb:lane * 2 + tb + 1],
                            op0=Alu.subtract, op1=Alu.mult)
                    xn_l.append(xn)

                # wave 2: transpose + evacuate (PE / DVE)
                xnT_l = []
                for lane in range(4):
                    s = 4 * g + lane
                    tr_ps = ps_a.tile([128, 512], BF16, tag="ps1",
                                      name=f"tr{s}")
                    for db in range(2):
                        for tb in range(2):
                            nc.tensor.transpose(
                                tr_ps[:, db * 256 + tb * 128:
                                      db * 256 + (tb + 1) * 128],
                                xn_l[lane][:, tb * 256 + db * 128:
                                           tb * 256 + (db + 1) * 128],
                                idt)
                    xnT = wk.tile([128, 512], BF16, tag="xnT", bufs=8,
                                  name=f"xnT{s}")
                    nc.vector.tensor_copy(xnT, tr_ps)
                    xnT_l.append(xnT)

                def proj_qk(name, s, w, xnT):
                    ps = ps_a.tile([128, 512], F32, tag="ps1",
                                   name=f"{name}ps{s}")
                    wt = const[f"w{name}_{w}"]
                    for odb in range(2):
                        sl = slice(odb * 256, (odb + 1) * 256)
                        for db in range(2):
                            nc.tensor.matmul(
                                ps[:, sl],
                                wt[:, db, odb * 128:(odb + 1) * 128],
                                xnT[:, db * 256:(db + 1) * 256],
                                start=(db == 0),
                                stop=(db == 1 and not with_bias))
                        if with_bias:
                            nc.tensor.matmul(
                                ps[:, sl],
                                const[f"b{name}_{w}"][:, odb * 128:(odb + 1) * 128],
                                ones_row, start=False, stop=True)
                    return ps

                def rope(src_sb, s, w, nm):
                    shuf = wk.tile([128, 512], BF16, tag=f"shuf{nm}",
                                   name=f"sh{nm}{s}")
                    nc.vector.stream_shuffle(shuf, src_sb, SWAP_MASK)
                    t2 = wk.tile([128, 512], BF16, tag=f"t2{nm}",
                                 name=f"t2{nm}{s}")
                    nc.vector.tensor_tensor(out=t2, in0=shuf,
                                            in1=const[f"sin_{w}"], op=Alu.mult)
                    t1 = wk.tile([128, 512], BF16, tag=f"t1{nm}",
                                 name=f"t1{nm}{s}")
                    nc.vector.tensor_tensor(out=t1, in0=src_sb,
                                            in1=const[f"cos_{w}"], op=Alu.mult)
                    eng = nc.gpsimd if nm == "q" else nc.vector
                    qr = wk.tile([128, 512], BF16, tag=f"qr{nm}", bufs=8,
                                 name=f"qr{nm}{s}")
                    eng.tensor_tensor(out=qr, in0=t1, in1=t2, op=Alu.add)
                    return qr

                # wave 3: q-proj + rope(q) per lane
                qr_l = []
                for lane in range(4):
                    s = 4 * g + lane
                    _, si, w, _, _ = seq_info(s)
                    q_ps = proj_qk("q", s, w, xnT_l[lane])
                    qc = wk.tile([128, 512], BF16, tag="qc", name=f"qc{s}")
                    nc.scalar.copy(qc, q_ps)
                    qr_l.append(rope(qc, s, w, "q"))

                # wave 4: k-proj + rope(k) per lane
                kr_l = []
                for lane in range(4):
                    s = 4 * g + lane
                    _, si, w, _, _ = seq_info(s)
                    k_ps = proj_qk("k", s, w, xnT_l[lane])
                    kc = wk.tile([128, 512], BF16, tag="kc", name=f"kc{s}")
                    nc.scalar.copy(kc, k_ps)
                    kr_l.append(rope(kc, s, w, "k"))

                grp_front[g] = (xnT_l, qr_l, kr_l)

            def back(g):
                """v proj -> scores -> exp -> sums -> AV for 4 seqs."""
                xnT_l, qr_l, kr_l = grp_front.pop(g)
                sums_ps = ps_u.tile([128, 1024], F32, tag="sums", name=f"sums{g}")
                nc.vector.memset(sums_ps, 1.0)
                grp_sums[g] = sums_ps

                # v-proj per lane, folded into the wave-6 stagger so the
                # first scores (and Act's exp chain) start immediately
                v_l = [None] * 4

                def v_proj(lane):
                    s = 4 * g + lane
                    _, si, w, _, _ = seq_info(s)
                    v_ps = ps_a.tile([128, 512], F32, tag="ps1", name=f"vps{s}")
                    for tb in range(2):
                        sl = slice(tb * 256, (tb + 1) * 256)
                        for db in range(2):
                            nc.tensor.matmul(
                                v_ps[:, sl],
                                xnT_l[lane][:, db * 256 + tb * 128:
                                            db * 256 + (tb + 1) * 128],
                                const[f"wv_{w}"][:, db, :],
                                start=(db == 0),
                                stop=(db == 1 and not with_bias))
                        if with_bias:
                            nc.tensor.matmul(v_ps[:, sl], ones_row[:, 0:128],
                                             const[f"bv_{w}"],
                                             start=False, stop=True)
                    v_sb = hp.tile([128, 512], BF16, tag="v_sb", bufs=4,
                                   name=f"vsb{s}")
                    nc.scalar.copy(v_sb, v_ps)
                    v_l[lane] = v_sb

                # wave 6, lane-pipelined: scores+exp for lane L overlap
                # sums+AV for lane L-1, so PE never waits on Act's exps
                def scores_exp(lane):
                    s = 4 * g + lane
                    qr, kr = qr_l[lane], kr_l[lane]
                    p_sb = []
                    for jb in range(2):
                        # two 1-bank score tiles per jb: hh0 heads (pc 0,1)
                        # and hh1 heads (pc 2,3) stay in separate banks
                        pp = [ps_s.tile([128, 512], F32, tag="psS",
                                        name=f"pps{s}_{jb}_{hh}")
                              for hh in range(2)]
                        for h in range(4):
                            odb, hh = divmod(h, 2)
                            off = hh * 64
                            nc.tensor.matmul(
                                pp[hh][:, odb * 256:(odb + 1) * 256],
                                kr[off:off + 64,
                                   odb * 256 + jb * 128:
                                   odb * 256 + (jb + 1) * 128],
                                qr[off:off + 64, odb * 256:(odb + 1) * 256],
                                start=True, stop=True)
                        psb = hp.tile([128, 1024], BF16, tag="p_sb", bufs=4,
                                      name=f"p{s}_{jb}")
                        for hh in range(2):
                            nc.scalar.activation(
                                psb[:, hh * 512:(hh + 1) * 512], pp[hh],
                                Act.Exp)
                        p_sb.append(psb)
                    return p_sb

                def sums_av(lane, p_sb):
                    s = 4 * g + lane
                    v_sb = v_l[lane]
                    for jb in range(2):
                        for half in range(2):
                            nc.tensor.matmul(
                                sums_ps[32 * lane:32 * lane + 1,
                                        half * 512:(half + 1) * 512],
                                ones_col,
                                p_sb[jb][:, half * 512:(half + 1) * 512],
                                start=(jb == 0), stop=(jb == 1),
                                tile_position=(0, 32 * lane))
                    o_ps = ps_a.tile([128, 512], F32, tag="ps1", name=f"ops{s}")
                    for h in range(4):
                        odb, hh = divmod(h, 2)
                        off = hh * 64
                        pc = PCOL[h]
                        for jb in range(2):
                            nc.tensor.matmul(
                                o_ps[off:off + 64, odb * 256:(odb + 1) * 256],
                                v_sb[:, jb * 256 + h * 64:
                                     jb * 256 + (h + 1) * 64],
                                p_sb[jb][:, pc * 256:(pc + 1) * 256],
                                start=(jb == 0), stop=(jb == 1))
                    o_sb = hp.tile([128, 512], BF16, tag="o_sb", bufs=8,
                                   name=f"osb{s}")
                    nc.vector.tensor_copy(o_sb, o_ps)
                    osb_s[s] = o_sb

                prev = None
                for lane in range(4):
                    cur = scores_exp(lane)
                    v_proj(lane)
                    if prev is not None:
                        sums_av(lane - 1, prev)
                    prev = cur
                sums_av(3, prev)

            def rec_grp(g):
                # group reciprocal + broadcast round trip (3 SWDGE descs),
                # deferred one pipeline stage so sums are long done and the
                # stats wave hides the round-trip latency before tail(g)
                sums_ps = grp_sums.pop(g)
                rec_f = wk.tile([128, 1024], F32, tag="rec_f", bufs=2, name=f"recf{g}")
                nc.vector.reciprocal_approx_fast(rec_f, sums_ps)
                rec_bf = wk.tile([128, 1024], BF16, tag="rec_bf", bufs=2,
                                 name=f"recbf{g}")
                nc.vector.tensor_copy(rec_bf, rec_f)
                pstr = rec_bf.ap[0][0]
                gat = bass.AP(tensor=rec_bf.tensor, offset=rec_bf.offset,
                              ap=[[32 * pstr, 4], [1, 1024]])
                nc.gpsimd.dma_start(out=rec_dram[g], in_=gat)
                recbc = hp.tile([128, 2048], BF16, tag="recbc", bufs=2,
                                name=f"recbc{g}")
                src0 = rec_dram[g]
                for hh in range(2):
                    src = bass.AP(tensor=src0.tensor,
                                  offset=src0.offset + hh * 512,
                                  ap=[[0, 64], [1024, 4], [1, 512]])
                    nc.gpsimd.dma_start(
                        out=recbc[hh * 64:(hh + 1) * 64, :].rearrange(
                            "p (l c) -> p l c", l=4),
                        in_=src)
                grp_rec[g] = recbc

            def tail(g):
                """normalize -> out-proj -> elu -> residual -> store."""
                recbc = grp_rec.pop(g)
                # wave A: normalize (DVE)
                on_l = []
                for lane in range(4):
                    s = 4 * g + lane
                    o_n = wk.tile([128, 512], BF16, tag="o_n", bufs=5,
                                  name=f"on{s}")
                    nc.vector.tensor_tensor(
                        out=o_n, in0=osb_s.pop(s),
                        in1=recbc[:, lane * 512:(lane + 1) * 512],
                        op=Alu.mult)
                    on_l.append(o_n)
                # wave B, lane-pipelined: out-proj(l) overlaps elu(l-1)
                def outproj(lane):
                    s = 4 * g + lane
                    _, si, w, _, _ = seq_info(s)
                    y_ps = ps_a.tile([128, 512], F32, tag="ps1",
                                     name=f"yps{s}")
                    for tb in range(2):
                        sl = slice(tb * 256, (tb + 1) * 256)
                        for odb in range(2):
                            nc.tensor.matmul(
                                y_ps[:, sl],
                                on_l[lane][:, odb * 256 + tb * 128:
                                           odb * 256 + (tb + 1) * 128],
                                const[f"wo_{w}"][:, odb, :],
                                start=(odb == 0),
                                stop=(odb == 1 and not with_bias))
                        if with_bias:
                            nc.tensor.matmul(y_ps[:, sl], ones_row[:, 0:128],
                                             const[f"bo_{w}"],
                                             start=False, stop=True)
                    return y_ps

                def elu_store(lane, y_ps):
                    s = 4 * g + lane
                    is_row, si, w, _, yout_d = seq_info(s)
                    xt = xt_s.pop(s)
                    # elu(t) = relu(t) + min(exp(t), 1) - 1   (exact, inf-safe)
                    E = wk.tile([128, 512], F32, tag="E", name=f"E{s}")
                    nc.scalar.activation(E, y_ps, Act.Exp)
                    ph = wk.tile([128, 512], F32, tag="ph", name=f"ph{s}")
                    nc.scalar.activation(ph, y_ps, Act.Relu, scale=0.5)
                    m = wk.tile([128, 512], F32, tag="m", name=f"m{s}")
                    nc.gpsimd.tensor_scalar(out=m, in0=E, scalar1=1.0,
                                            scalar2=0.5, op0=Alu.min,
                                            op1=Alu.mult)
                    acc = wk.tile([128, 512], F32, tag="acc", name=f"acc{s}")
                    nc.vector.scalar_tensor_tensor(out=acc, in0=m, scalar=-0.5,
                                                   in1=ph, op0=Alu.add,
                                                   op1=Alu.add)
                    if is_row:
                        yf = wk.tile([128, 512], F32, tag="yf", name=f"yf{s}")
                        nc.gpsimd.tensor_tensor(out=yf, in0=acc, in1=xt,
                                                op=Alu.add)
                    else:
                        yf = acc
                    nc.sync.dma_start(
                        out=yout_d[si].rearrange("(a p) d -> p a d", p=128),
                        in_=yf.rearrange("p (a d) -> p a d", a=2))

                prev_y = None
                for lane in range(4):
                    y = outproj(lane)
                    if prev_y is not None:
                        elu_store(lane - 1, prev_y)
                    prev_y = y
                elu_store(3, prev_y)

            for it in range(n_grp + 4):
                if it < n_grp:
                    load_grp(it)
                if it >= 4:
                    rec_grp(it - 4)
                if 1 <= it <= n_grp:
                    stats_grp(it - 1)
                if it >= 4:
                    tail(it - 4)
                if 3 <= it <= n_grp + 2:
                    back(it - 3)
                if 2 <= it <= n_grp + 1:
                    front(it - 2)

    nc.finalize()
    return nc


_NC_CACHE = {}


def _get_nc(n_row, n_col, with_bias=False):
    key = (n_row, n_col, with_bias)
    if key not in _NC_CACHE:
        _NC_CACHE[key] = _build_nc(n_row, n_col, with_bias)
    return _NC_CACHE[key]


def _prep_consts(sin_i, cos_i, sin_j, cos_j,
                 gia, bia, gib, bib, Wq_i, Wkv_i, Wo_i, bo_i,
                 gja, bja, gjb, bjb, Wq_j, Wkv_j, Wo_j, bo_j):
    def fold(g_a, b_a, g_b, b_b, Wq, Wkv, Wo, bo, sin, cos):
        Wq = np.asarray(Wq, np.float32)
        Wkv = np.asarray(Wkv, np.float32)
        Wo = np.asarray(Wo, np.float32)
        g_a = np.asarray(g_a, np.float32); b_a = np.asarray(b_a, np.float32)
        g_b = np.asarray(g_b, np.float32); b_b = np.asarray(b_b, np.float32)
        wq = (g_a[:, None] * Wq)
        bq = b_a @ Wq
        wk = (g_b[:, None] * Wkv[:, :256]); bk = b_b @ Wkv[:, :256]
        wv = (g_b[:, None] * Wkv[:, 256:]); bv = b_b @ Wkv[:, 256:]
        # out features are interleaved (d h): permute Wo rows to head-blocked
        perm = (np.arange(IDIM)[None, :] * HEADS
                + np.arange(HEADS)[:, None]).reshape(-1)
        wo = Wo[perm, :]
        sin = np.asarray(sin, np.float32)[0]   # [256, 64]
        cos = np.asarray(cos, np.float32)[0]
        p = np.arange(128)
        sgn = np.where(p % 2 == 0, -1.0, 1.0).astype(np.float32)
        sinT = sgn[:, None] * sin[:, p % 64].T       # [128, 256]
        cosT = cos[:, p % 64].T                      # [128, 256]
        return dict(
            wq=wq.reshape(2, 128, 256).astype(BF),
            wk=wk.reshape(2, 128, 256).astype(BF),
            wv=wv.reshape(2, 128, 256).astype(BF),
            wo=wo.reshape(2, 128, 256).astype(BF),
            bq=bq.reshape(1, 256).astype(BF), bk=bk.reshape(1, 256).astype(BF),
            bv=bv.reshape(1, 256).astype(BF),
            bo=np.asarray(bo, np.float32).reshape(1, 256).astype(BF),
            cos=np.tile(cosT, (1, 2)).astype(BF),    # [128, 512] odb-duplicated
            sin=np.tile(sinT, (1, 2)).astype(BF),
        )

    ca = fold(gia, bia, gib, bib, Wq_i, Wkv_i, Wo_i, bo_i, sin_i, cos_i)
    cb = fold(gja, bja, gjb, bjb, Wq_j, Wkv_j, Wo_j, bo_j, sin_j, cos_j)
    consts = {}
    for w, c in (("a", ca), ("b", cb)):
        for k, v in c.items():
            consts[f"{k}_{w}"] = v
    r2 = np.zeros((128, 128), np.float32)
    mm = np.arange(128)
    r2[mm ^ 1, mm] = 1.0
    consts["r2"] = r2.astype(BF)
    consts["idt"] = np.eye(128, dtype=np.float32).astype(BF)
    return consts


def kernel(x, sin_i, cos_i, sin_j, cos_j,
           gia, bia, gib, bib, Wq_i, Wkv_i, Wo_i, bo_i,
           gja, bja, gjb, bjb, Wq_j, Wkv_j, Wo_j, bo_j):
    x = np.asarray(x, np.float32)
    consts = _prep_consts(sin_i, cos_i, sin_j, cos_j,
                          gia, bia, gib, bib, Wq_i, Wkv_i, Wo_i, bo_i,
                          gja, bja, gjb, bjb, Wq_j, Wkv_j, Wo_j, bo_j)
    with_bias = any(
        np.abs(np.asarray(consts[f"{nm}_{w}"], np.float32)).max() > 0
        for nm in ("bq", "bk", "bv", "bo") for w in ("a", "b"))
    if not with_bias:
        for nm in ("bq", "bk", "bv", "bo"):
            for w in ("a", "b"):
                del consts[f"{nm}_{w}"]
    nc = _get_nc(NROW, NCOL, with_bias)

    xg = x[0]                                    # [I, J, D]
    xt = np.ascontiguousarray(xg.transpose(1, 0, 2))   # [J, I, D]
    in_maps = []
    for c in range(NCORES):
        m = dict(consts)
        m["xr"] = np.ascontiguousarray(xg[c * NROW:(c + 1) * NROW])
        m["xc"] = np.ascontiguousarray(xt[c * NCOL:(c + 1) * NCOL])
        in_maps.append(m)

    res = run_bass_kernel_spmd(nc, in_maps, list(range(NCORES)))

    out = np.empty((1, I, J, DIM), np.float32)
    for c in range(NCORES):
        out[0, c * NROW:(c + 1) * NROW] = res.results[c]["yr"]
    for c in range(NCORES):
        out[0, :, c * NCOL:(c + 1) * NCOL, :] += \
            res.results[c]["yc"].transpose(1, 0, 2)
    return out
